# revision 29
# baseline (speedup 1.0000x reference)
"""Causal multi-head self-attention (B=4, S=2048, D=1024, H=16) on 8 Trainium2
NeuronCores.

Sharding: batch x head-group. Core c handles batch b = c//2 and head group
g = c%2 (8 of the 16 heads). Each core computes the full attention for its
(b, g) shard plus the partial output projection over its 512 attention-output
features; the host sums the two partial projections per batch element.

v2 (all-bf16 inputs, fp32 PSUM accumulation everywhere):
  - All matmul operands are bf16 (x^T, w_qkv^T, w_out^T pre-transposed and
    bf16-rounded on the host): halves weight/activation DMA vs f32r, enables
    fast-weight-load, and keeps the PE's HAM clock warm by shortening the
    DMA-paced startup.
  - Q^T is SBUF-resident (no DRAM spill/reload).
  - QKV projection: Q^T/K^T feature-major, 2 heads per 128-partition tile;
    V seq-major.  Startup interleaves wq column-block DMAs with the first
    seq-slice's projection chunks so the PE starts ~3us in.
  - Attention per head pair: S^T = K^T.T @ Q^T with K=64 row-tiled pairs
    (both heads concurrent in the PE array via tile_position auto-derive,
    outputs side by side in one 2-bank PSUM tile), one exp per kv-tile on
    ACT, causal diagonal handled by a post-exp multiply with a 0/1 triangle
    (bf16 2x DVE) instead of additive -1e9 masking.
  - AV: head A's stationary is [V_A | ones] -> out partitions 0-63 + denom
    at 64; head B's stationary is [ones | 0..0 | V_B] -> denom at partition
    0 + out partitions 64-127, so the normalized B half writes straight to
    aot[64:128] with no partition-shift DMA.
  - Normalization: fast reciprocal straight from the PSUM denominator rows,
    one broadcast each for A (partitions 0-63) and B (0-127), two multiplies.
  - Output projection y = aot.T @ w_out^T accumulated over head pairs,
    emitted as filler during the last query block (the ACT-bound phase).
"""

import sys

if "/opt/trn_rl_repo" not in sys.path:
    sys.path.insert(0, "/opt/trn_rl_repo")

import numpy as np

BATCH = 4
SEQ = 2048
D = 1024
HEADS = 16
HD = 64
N_CORES = 8
HPC = 8          # heads per core
PAIRS = HPC // 2
KT_D = D // 128  # contraction tiles over d_model
SEQ_T = SEQ // 128
QB = SEQ // 512  # query blocks of 512

_CACHED = {}


def _build_nc():
    import concourse.bass as bass  # noqa: F401
    import concourse.tile as tile
    from concourse import bacc, mybir

    f32 = mybir.dt.float32
    bf16 = mybir.dt.bfloat16
    EXP = mybir.ActivationFunctionType.Exp

    nc = bacc.Bacc("TRN2", target_bir_lowering=False, debug=False,
                   num_devices=N_CORES)

    xt_d = nc.dram_tensor("xt", [D, SEQ], bf16, kind="ExternalInput").ap()
    wq_d = nc.dram_tensor("wq", [D, 1536], bf16, kind="ExternalInput").ap()
    wo_d = nc.dram_tensor("wo", [512, D], bf16, kind="ExternalInput").ap()
    tri_d = nc.dram_tensor("tri", [128, 256], bf16, kind="ExternalInput").ap()
    ones_d = nc.dram_tensor("onescol", [128, 64], bf16, kind="ExternalInput").ap()
    vpb0_d = nc.dram_tensor("vpb0", [128, 256], bf16, kind="ExternalInput").ap()
    y_d = nc.dram_tensor("y", [SEQ, D], f32, kind="ExternalOutput").ap()
    scr_d = nc.dram_tensor("scr", [128, 16], f32, kind="ExternalOutput").ap()

    xt_t = xt_d.rearrange("(k p) s -> p k s", p=128)
    wq_t = wq_d.rearrange("(k p) f -> p k f", p=128)
    wo_t = wo_d.rearrange("(k p) f -> p k f", p=128)

    with tile.TileContext(nc) as tc:
        with tc.tile_pool(name="persist", bufs=1) as persist, \
             tc.tile_pool(name="xts", bufs=2) as xts_pool, \
             tc.tile_pool(name="pt", bufs=6) as pt_pool, \
             tc.tile_pool(name="small", bufs=3) as small, \
             tc.tile_pool(name="wsp", bufs=20) as wspool, \
             tc.tile_pool(name="rcp", bufs=2) as rcp, \
             tc.tile_pool(name="pss", bufs=2, space="PSUM") as ps_s, \
             tc.tile_pool(name="psf", bufs=2, space="PSUM") as ps_f, \
             tc.tile_pool(name="psa", bufs=1, space="PSUM") as ps_a, \
             tc.tile_pool(name="psb", bufs=1, space="PSUM") as ps_b:

            # warm-up burst first: a memset tile (no DMA dependency — DMA
            # rings take ~5us to spin up) feeds a long matmul accumulation
            # that keeps the PE busy until real weights arrive, so the HAM
            # clock gate opens to 8/8 before the first real matmul.  The
            # result reaches an external output so nothing is eliminated.
            wtile = persist.tile([128, 256], bf16, tag="wtile")
            nc.vector.memset(wtile[:], 1.0)
            wps = ps_f.tile([128, 512], f32, tag="f", name="warmps")
            for i in range(18):
                nc.tensor.matmul(wps[:, 0:256], wtile[:, 0:128], wtile[:],
                                 start=(i == 0), stop=(i == 17))
            # consume the burst so it isn't dead-code-eliminated, but defer
            # the scratch DMA to the end of emission: an early DMA that waits
            # on the burst would head-of-line-block the queue for the real
            # weight/activation loads
            deferred_scr = []
            wsb = small.tile([128, 8], f32, tag="wsb", name="wsb")
            nc.vector.tensor_copy(wsb[:], wps[:, 0:8])
            deferred_scr.append((wsb, 0, 8))

            # ---- persistent SBUF tensors ----
            wq = persist.tile([128, KT_D, 1536], bf16, tag="wq")
            wo = persist.tile([128, 4, D], bf16, tag="wo")
            tri = persist.tile([128, 256], bf16, tag="tri")
            onescol = persist.tile([128, 64], bf16, tag="onescol")
            vpb0 = persist.tile([128, 256], bf16, tag="vpb0")

            kt = [persist.tile([128, SEQ], bf16, tag=f"kt{p}", name=f"kt{p}")
                  for p in range(PAIRS)]
            qt = [persist.tile([128, SEQ], bf16, tag=f"qt{p}", name=f"qt{p}")
                  for p in range(PAIRS)]
            aot = [persist.tile([128, SEQ], bf16, tag=f"aot{p}", name=f"aot{p}")
                   for p in range(PAIRS)]
            # AV stationaries: A = [hd 0:64 | ones col 64]; B = [ones col 0 |
            # zeros 1:64 | hd 64:128]
            vpA = persist.tile([128, SEQ_T, PAIRS, HD + 1], bf16, tag="vpA")
            vpB = persist.tile([128, SEQ_T, PAIRS, 128], bf16, tag="vpB")

            nc.sync.dma_start(out=tri[:], in_=tri_d[:])
            nc.sync.dma_start(out=onescol[:], in_=ones_d[:])
            nc.sync.dma_start(out=vpb0[:], in_=vpb0_d[:])
            triv = tri.rearrange("p (s c) -> p s c", s=2)

            # one-time constant init of vpA ones column and vpB [ones|zeros]
            # columns (DVE is idle at startup)
            nc.vector.tensor_copy(
                vpA.rearrange("p s q e -> p (s q) e")[:, :, HD],
                onescol[:])
            for st in range(SEQ_T):
                nc.vector.tensor_copy(
                    vpB[:, st, :, 0:HD],
                    vpb0.rearrange("p (q e) -> p q e", q=PAIRS))

            # touch exp so the ACT table set loads during the DMA-paced
            # startup instead of at the first attention step
            warm = small.tile([1, 64], f32, tag="warm", name="warm")
            nc.scalar.activation(out=warm[:], in_=onescol[0:1, :], func=EXP)

            # warm filler chunk: ~0.9us of dummy PE work used to keep the
            # clock gate open through the ACT-bound last query block
            warm_n = [0]

            def gen_warm_chunk():
                def chunk():
                    ps = ps_f.tile([128, 512], f32, tag="f", name="wfill")
                    for i in range(4):
                        nc.tensor.matmul(ps[:, 0:256], wtile[:, 0:128],
                                         wtile[:], start=(i == 0), stop=(i == 3))
                    col = 8 + warm_n[0] % 8
                    warm_n[0] += 1
                    ws = wspool.tile([128, 1], f32, tag="ws1", name="ws1")
                    nc.vector.tensor_copy(ws[:], ps[:, 0:1])
                    deferred_scr.append((ws, col, col + 1))
                return chunk

            # ---- projection chunks ----
            def gen_proj_chunks(s):
                """Projection work for seq slice s (512 wide) as ~2us PE
                chunks, drained between attention steps as filler."""
                c = s * 512
                state = {}

                def load():
                    with nc.named_scope("qkv_proj"):
                        xts = xts_pool.tile([128, KT_D, 512], bf16, tag="xts",
                                            name="xts")
                        nc.sync.dma_start(out=xts[:],
                                          in_=xt_t[:, :, c:c + 512])
                        state["xts"] = xts

                def qk(p, qkx):
                    def chunk():
                        with nc.named_scope("qkv_proj"):
                            xts = state["xts"]
                            f0 = p * 256 + qkx * 128
                            ps = ps_f.tile([128, 512], f32, tag="f", name="ps")
                            for k in range(KT_D):
                                nc.tensor.matmul(ps[:], wq[:, k, f0:f0 + 128],
                                                 xts[:, k, :],
                                                 start=(k == 0), stop=(k == KT_D - 1))
                            dst = qt[p] if qkx == 0 else kt[p]
                            nc.vector.tensor_copy(dst[:, c:c + 512], ps[:])
                    return chunk

                def vproj(t):
                    def chunk():
                        with nc.named_scope("qkv_proj"):
                            xts = state["xts"]
                            st = s * 4 + t
                            psv = ps_f.tile([128, 512], f32, tag="f", name="psv")
                            for k in range(KT_D):
                                nc.tensor.matmul(psv[:],
                                                 xts[:, k, t * 128:(t + 1) * 128],
                                                 wq[:, k, 1024:1536],
                                                 start=(k == 0), stop=(k == KT_D - 1))
                            pv = psv.rearrange("p (q two e) -> p q two e",
                                               q=PAIRS, two=2)
                            nc.vector.tensor_copy(vpA[:, st, :, 0:HD],
                                                  pv[:, :, 0, :])
                            nc.vector.tensor_copy(vpB[:, st, :, HD:128],
                                                  pv[:, :, 1, :])
                    return chunk

                chunks = [load]
                for p in range(PAIRS):
                    chunks.append(qk(p, 0))
                    chunks.append(qk(p, 1))
                for t in range(4):
                    chunks.append(vproj(t))
                return chunks

            def emit_slice0():
                """First seq slice with wq column-block DMAs interleaved so
                the first projection matmuls start ~3us in."""
                with nc.named_scope("qkv_proj"):
                    xts = xts_pool.tile([128, KT_D, 512], bf16, tag="xts",
                                        name="xts")
                    state = {"xts": xts}

                    def qk(p, qkx):
                        f0 = p * 256 + qkx * 128
                        ps = ps_f.tile([128, 512], f32, tag="f", name="ps")
                        for k in range(KT_D):
                            nc.tensor.matmul(ps[:], wq[:, k, f0:f0 + 128],
                                             xts[:, k, :],
                                             start=(k == 0), stop=(k == KT_D - 1))
                        dst = qt[p] if qkx == 0 else kt[p]
                        nc.vector.tensor_copy(dst[:, 0:512], ps[:])

                    nc.sync.dma_start(out=xts[:], in_=xt_t[:, :, 0:512])
                    nc.sync.dma_start(out=wq[:, :, 0:256],
                                      in_=wq_t[:, :, 0:256])
                    qk(0, 0)
                    qk(0, 1)
                    for p in range(1, PAIRS):
                        nc.sync.dma_start(out=wq[:, :, p * 256:(p + 1) * 256],
                                          in_=wq_t[:, :, p * 256:(p + 1) * 256])
                        qk(p, 0)
                        qk(p, 1)
                    nc.sync.dma_start(out=wq[:, :, 1024:1536],
                                      in_=wq_t[:, :, 1024:1536])
                    for t in range(4):
                        psv = ps_f.tile([128, 512], f32, tag="f", name="psv")
                        for k in range(KT_D):
                            nc.tensor.matmul(psv[:],
                                             xts[:, k, t * 128:(t + 1) * 128],
                                             wq[:, k, 1024:1536],
                                             start=(k == 0), stop=(k == KT_D - 1))
                        pv = psv.rearrange("p (q two e) -> p q two e",
                                           q=PAIRS, two=2)
                        nc.vector.tensor_copy(vpA[:, t, :, 0:HD], pv[:, :, 0, :])
                        nc.vector.tensor_copy(vpB[:, t, :, HD:128], pv[:, :, 1, :])

            # ---- attention per (pair, query block) ----
            def emit_unit_group(pg, qb, on_step):
                with nc.named_scope("attention"):
                    q0 = qb * 512
                    n_kv = (qb + 1) * 4
                    aoA = ps_a.tile([65, 512], f32, tag="aoA", name="aoA")
                    aoB = ps_b.tile([128, 512], f32, tag="aoB", name="aoB")
                    pts = {}

                    def emit_scores(j):
                        delta = j * 128 - q0
                        c0 = max(delta, 0)
                        kv = j * 128
                        sps = ps_s.tile([128, 1024], f32, tag="s", name="sps")
                        # head A at cols [c0, 512), head B left-aligned at
                        # [512, 1024-c0) so the exp range is contiguous
                        nc.tensor.matmul(sps[:, c0:512],
                                         kt[pg][0:64, kv:kv + 128],
                                         qt[pg][0:64, q0 + c0:q0 + 512],
                                         start=True, stop=True)
                        nc.tensor.matmul(sps[:, 512:1024 - c0],
                                         kt[pg][64:128, kv:kv + 128],
                                         qt[pg][64:128, q0 + c0:q0 + 512],
                                         start=True, stop=True)
                        pt = pt_pool.tile([128, 1024], bf16, tag="pt", name="pt")
                        nc.scalar.activation(out=pt[:, c0:1024 - c0],
                                             in_=sps[:, c0:1024 - c0], func=EXP)
                        if delta >= 0:
                            ptA = pt[:, c0:c0 + 128]
                            nc.vector.tensor_mul(ptA, ptA, triv[:, 0, :])
                            ptB = pt[:, 512:640]
                            nc.vector.tensor_mul(ptB, ptB, triv[:, 1, :])
                        pts[j] = pt

                    def emit_av(j):
                        delta = j * 128 - q0
                        c0 = max(delta, 0)
                        pt = pts.pop(j)
                        nc.tensor.matmul(aoA[:, c0:512], vpA[:, j, pg, :],
                                         pt[:, c0:512],
                                         start=(j == 0), stop=(j == n_kv - 1))
                        nc.tensor.matmul(aoB[:, c0:512], vpB[:, j, pg, :],
                                         pt[:, 512:1024 - c0],
                                         start=(j == 0), stop=(j == n_kv - 1))

                    # kv steps in pairs: [S jj, S jj+1] run in 64x128 tiled
                    # mode back-to-back, then [AV jj-2, AV jj-1] in full-array
                    # mode — halves the PE tiling-mode switch drains
                    for jj in range(0, n_kv, 2):
                        emit_scores(jj)
                        emit_scores(jj + 1)
                        if jj >= 2:
                            emit_av(jj - 2)
                            emit_av(jj - 1)
                        on_step()
                        on_step()
                    emit_av(n_kv - 2)
                    emit_av(n_kv - 1)

                    # normalization: denom A at aoA partition 64, denom B at
                    # aoB partition 0.  Copy both rows to SBUF partition 0
                    # (reciprocal_approx_fast misreads PSUM sources on HW),
                    # one reciprocal, gpsimd broadcast, multiply.
                    dA = small.tile([65, 512], f32, tag="dA", name="dA")
                    nc.vector.tensor_copy(dA[64:65, :], aoA[64:65, :])
                    d0 = small.tile([1, 1024], f32, tag="d0", name="d0")
                    nc.sync.dma_start(out=d0[:, 0:512], in_=dA[64:65, :])
                    nc.vector.tensor_copy(d0[0:1, 512:1024], aoB[0:1, :])
                    rr = small.tile([1, 1024], f32, tag="rr", name="rr")
                    nc.vector.reciprocal_approx_fast(out=rr[:], in_=d0[:])
                    rcA = rcp.tile([64, 512], f32, tag="rcA", name="rcA")
                    rcB = rcp.tile([128, 512], f32, tag="rcB", name="rcB")
                    nc.gpsimd.partition_broadcast(rcA[:], rr[0:1, 0:512])
                    nc.gpsimd.partition_broadcast(rcB[:], rr[0:1, 512:1024])
                    nc.vector.tensor_mul(aot[pg][0:64, q0:q0 + 512],
                                         aoA[0:64, :], rcA[:])
                    nc.vector.tensor_mul(aot[pg][64:128, q0:q0 + 512],
                                         aoB[64:128, :], rcB[64:128, :])

            # ---- output projection chunks (partial; host sums groups) ----
            def wo_load_chunk():
                with nc.named_scope("out_proj"):
                    nc.sync.dma_start(out=wo[:], in_=wo_t[:])

            def gen_outproj_chunk(st, do):
                def chunk():
                    with nc.named_scope("out_proj"):
                        r = st * 128
                        c = do * 512
                        py = ps_f.tile([128, 512], f32, tag="f", name="py")
                        for p in range(PAIRS):
                            nc.tensor.matmul(py[:], aot[p][:, r:r + 128],
                                             wo[:, p, c:c + 512],
                                             start=(p == 0), stop=(p == PAIRS - 1))
                        ysb = small.tile([128, 512], f32, tag="ysb", name="ysb")
                        nc.vector.tensor_copy(ysb[:], py[:])
                        nc.sync.dma_start(out=y_d[r:r + 128, c:c + 512], in_=ysb[:])
                return chunk

            # ---- interleaved schedule ----
            from collections import deque

            emit_slice0()
            for qb in range(QB):
                queue = deque()
                if qb < QB - 1:
                    queue.extend(gen_proj_chunks(qb + 1))
                    if qb == QB - 2:
                        queue.appendleft(wo_load_chunk)
                else:
                    for oqb in range(QB - 1):
                        for st in range(oqb * 4, oqb * 4 + 4):
                            for do in range(2):
                                queue.append(gen_outproj_chunk(st, do))
                    for _ in range(10):
                        queue.append(gen_warm_chunk())
                steps_total = 16 * (qb + 1)
                q0len = len(queue)
                state = [0, 0]  # step index, chunks emitted

                def on_step():
                    # Bresenham pacing: spread the queue evenly across ALL
                    # steps of this query block (ceil-based draining would
                    # exhaust the queue early and starve the last pairs)
                    state[0] += 1
                    target = (state[0] * q0len) // steps_total
                    if state[0] >= steps_total:
                        target = q0len
                    while state[1] < target and queue:
                        queue.popleft()()
                        state[1] += 1

                for pg in range(PAIRS):
                    emit_unit_group(pg, qb, on_step)
                while queue:
                    queue.popleft()()

            # warm filler across the last pair's normalization latency
            for _ in range(8):
                gen_warm_chunk()()

            # tail: output projection for the last query-block row
            for st in range(12, 16):
                for do in range(2):
                    gen_outproj_chunk(st, do)()

            # deferred scratch DMAs (burst/warm-chunk consumers)
            for ws, c0_, c1_ in deferred_scr:
                nc.sync.dma_start(out=scr_d[:, c0_:c1_], in_=ws[:, 0:c1_ - c0_])

    nc.compile()
    return nc


def _get_nc():
    if "nc" not in _CACHED:
        _CACHED["nc"] = _build_nc()
    return _CACHED["nc"]


def _make_in_maps(x, w_qkv, w_out):
    import ml_dtypes
    bf16 = ml_dtypes.bfloat16

    x = np.asarray(x, dtype=np.float32)
    w_qkv = np.asarray(w_qkv, dtype=np.float32)
    w_out = np.asarray(w_out, dtype=np.float32)

    xts = [np.ascontiguousarray(x[b].T).astype(bf16) for b in range(BATCH)]

    wqs, wos = [], []
    for g in range(2):
        W = np.empty((D, 1536), dtype=np.float32)
        for p in range(PAIRS):
            h0 = g * HPC + 2 * p
            W[:, p * 256:p * 256 + 128] = w_qkv[h0 * HD:h0 * HD + 128].T * 0.125
            W[:, p * 256 + 128:p * 256 + 256] = w_qkv[D + h0 * HD:D + h0 * HD + 128].T
        W[:, 1024:1536] = w_qkv[2 * D + g * 512:2 * D + (g + 1) * 512].T
        wqs.append(W.astype(bf16))
        wos.append(np.ascontiguousarray(w_out[:, g * 512:(g + 1) * 512].T).astype(bf16))

    tri01 = (np.arange(128)[None, :] >= np.arange(128)[:, None]).astype(np.float32)
    tri = np.concatenate([tri01, tri01], axis=1).astype(bf16)
    onescol = np.ones((128, 64), dtype=np.float32).astype(bf16)
    vpb0 = np.zeros((128, 256), dtype=np.float32)
    vpb0[:, 0::64] = 1.0
    vpb0 = vpb0.astype(bf16)

    in_maps = []
    for c in range(N_CORES):
        b, g = c // 2, c % 2
        in_maps.append({"xt": xts[b], "wq": wqs[g], "wo": wos[g],
                        "tri": tri, "onescol": onescol, "vpb0": vpb0})
    return in_maps


def kernel(x, w_qkv, w_out, _trace=False):
    from concourse.bass_utils import run_bass_kernel_spmd

    nc = _get_nc()
    in_maps = _make_in_maps(x, w_qkv, w_out)
    res = run_bass_kernel_spmd(nc, in_maps, list(range(N_CORES)), trace=_trace)
    _CACHED["last_results"] = res

    y = np.empty((BATCH, SEQ, D), dtype=np.float32)
    for b in range(BATCH):
        y[b] = res.results[2 * b]["y"] + res.results[2 * b + 1]["y"]
    return y
